# revision 11
# baseline (speedup 1.0000x reference)
"""Trainium2 Bass kernel for nn_BinancePerpStructuralLoss.

loss = sum_{t,c} mean_b relu(pred[b,t,idx_a[c]] - pred[b,t,idx_b[c]])
     = (1/B) * sum_{b,t,c} relu(pred[b,t,idx_a[c]] - pred[b,t,idx_b[c]])

Strategy (memory-bound problem, 126 MB of predictions):
  - Data-parallel: shard batch (128) across 8 cores -> 16 batches/core,
    i.e. a contiguous [16384, 240] row block per core.
  - Stream rows through SBUF in supertiles of [128 partitions x RPP rows
    x 240 features] (~2 MB per DMA, HWDGE, triple-buffered).
  - relu(a-b) = max(a,b) - b splits the loss into two LINEAR reductions,
    each computable by a fused single-pass engine op:
      * VectorE scalar_tensor_tensor(bypass, max, accum_out):
            accum = sum max(xa, xb)          (one DVE pass)
      * ScalarE activation(Copy, accum_out):
            accum = sum xb                   (one ACT pass)
    Both run over strided access-pattern views of the SBUF tile, baked at
    kernel-build time from the runtime idx_a/idx_b vectors (the index
    structure is periodic per book pair: runs of stride-1 constraints).
  - Per-core partial sums [128, slots] are DMA'd out; the final tiny
    reduction (sum, subtract, /128) happens on host in f64.

Raw Block-based bass (no TileContext): the installed walrus rejects
Tile's multi-wait tail drain and the InstTensorTensorReduce encoding.
"""

import sys

for _p in ("/opt/trn_rl_repo",):
    if _p not in sys.path:
        sys.path.insert(0, _p)

import numpy as np

import concourse.bass as bass
from concourse import mybir
from concourse.bass_utils import run_bass_kernel_spmd

# Problem shape (hardcoded per task contract).
B, T, F = 128, 1024, 240
NCORES = 8
BPC = B // NCORES            # batches per core = 16
ROWS = BPC * T               # rows per core = 16384
P = 128                      # SBUF partitions
RPP = 16                     # rows per partition per supertile
S = ROWS // (P * RPP)        # supertiles per core = 8
FREE = RPP * F               # free elems per partition per supertile
NBUF = 3                     # input tile buffers

f32 = mybir.dt.float32


# ---------------------------------------------------------------------------
# Index-structure decomposition: express the gather as a few affine views.
# ---------------------------------------------------------------------------

def _find_period(a, b):
    """Smallest plen such that a/b repeat with constant shifts every plen."""
    C = len(a)
    for plen in range(1, C // 2 + 1):
        if C % plen:
            continue
        n = C // plen
        aa = a.reshape(n, plen)
        bb = b.reshape(n, plen)
        da = np.diff(aa, axis=0)
        db = np.diff(bb, axis=0)
        if (da == da[0, 0]).all() and (db == db[0, 0]).all():
            return n, plen, int(da[0, 0]), int(db[0, 0])
    return 1, C, 0, 0


def _runs(a, b):
    """Split one period into maximal constant-stride runs.

    Returns list of (start, length, da, db); singletons get stride 0.
    """
    L = len(a)
    if L == 1:
        return [(0, 1, 0, 0)]
    dA = np.diff(a)
    dB = np.diff(b)
    delta_runs = []  # (first_delta, n_deltas, da, db)
    i = 0
    while i < L - 1:
        j = i
        while j + 1 < L - 1 and dA[j + 1] == dA[i] and dB[j + 1] == dB[i]:
            j += 1
        delta_runs.append((i, j - i + 1, int(dA[i]), int(dB[i])))
        i = j + 1
    # A delta-run over deltas [s, s+n) covers elements [s, s+n]. Adjacent
    # runs share one boundary element; give it to the longer run.
    claimed = [False] * L
    out = []
    for (s, n, da, db) in sorted(delta_runs, key=lambda r: -r[1]):
        lo, hi = s, s + n
        while lo <= hi and claimed[lo]:
            lo += 1
        while hi >= lo and claimed[hi]:
            hi -= 1
        if hi - lo + 1 >= 2:
            for e in range(lo, hi + 1):
                claimed[e] = True
            out.append((lo, hi - lo + 1, da, db))
    for e in range(L):
        if not claimed[e]:
            out.append((e, 1, 0, 0))
    out.sort()
    return out


def _groups(idx_a, idx_b):
    """Decompose (idx_a, idx_b) into 2-level affine groups."""
    nper, plen, psa, psb = _find_period(idx_a, idx_b)
    runs = _runs(idx_a[:plen], idx_b[:plen])
    gs = []
    for (s0, ln, da, db) in runs:
        gs.append(dict(
            off_a=int(idx_a[s0]), off_b=int(idx_b[s0]),
            nper=nper, psa=psa, psb=psb,
            ln=ln, ra=da, rb=db,
        ))
    # Safety: groups must cover each (a, b) pair exactly once (any order).
    got = []
    for g in gs:
        for q in range(g["nper"]):
            for k in range(g["ln"]):
                got.append((g["off_a"] + q * g["psa"] + k * g["ra"],
                            g["off_b"] + q * g["psb"] + k * g["rb"]))
    want = sorted(zip(idx_a.tolist(), idx_b.tolist()))
    if sorted(got) != want:
        # Fallback: one singleton group per constraint (correct, slower).
        gs = [dict(off_a=int(a), off_b=int(b), nper=1, psa=0, psb=0,
                   ln=1, ra=0, rb=0)
              for a, b in zip(idx_a.tolist(), idx_b.tolist())]
    return gs


def _coalesce(dims):
    """Merge adjacent [step, count] dims when outer step == inner step*count."""
    dims = [d for d in dims if d[1] != 1]
    if not dims:
        return [[1, 1]]
    out = [dims[0]]
    for st, c in dims[1:]:
        pst, pc = out[-1]
        if pst == st * c:
            out[-1] = [st, pc * c]
        else:
            out.append([st, c])
    return out


def _view(ap, extra_off, dims):
    """Strided free-dim view of SBUF AP `ap` (partition dim kept)."""
    pstep, pcount = ap.ap[0]
    return bass.AP(ap.tensor, ap.offset + extra_off, [[pstep, pcount]] + dims)


def _contig_dims(counts):
    dims = []
    stride = 1
    for c in reversed(counts):
        dims.append([stride, c])
        stride *= c
    return list(reversed(dims))


# ---------------------------------------------------------------------------
# Bass program (single core; run SPMD on 8 cores with different shards).
# ---------------------------------------------------------------------------

def _build(groups, reps=1):
    """Build the per-core program. reps>1 repeats the whole pipeline
    back-to-back inside one NEFF (benchmarking: amortizes host dispatch)."""
    G = len(groups)
    nc = bass.Bass()
    x = nc.declare_dram_parameter("x", [ROWS, F], f32, isOutput=False)
    out_mx = nc.declare_dram_parameter("out_mx", [P, S * G], f32, isOutput=True)
    out_b = nc.declare_dram_parameter("out_b", [P, S * G], f32, isOutput=True)

    xv = x.rearrange("(s p r) f -> s p (r f)", p=P, r=RPP)

    # Per-group AP fragments (free dims over one supertile tile).
    gdims = []
    scr_off = []  # scratch slice offset per group
    scr_total = 0
    for g in groups:
        da = _coalesce([[F, RPP], [g["psa"], g["nper"]], [g["ra"], g["ln"]]])
        db = _coalesce([[F, RPP], [g["psb"], g["nper"]], [g["rb"], g["ln"]]])
        counts = [c for (_, c) in da]
        nelem = int(np.prod(counts))
        scr_off.append(scr_total)
        scr_total += nelem
        gdims.append((g["off_a"], da, g["off_b"], db, _contig_dims(counts)))

    import contextlib

    with contextlib.ExitStack() as ctx:
        xt = ctx.enter_context(nc.sbuf_tensor([P, NBUF * FREE], f32))
        scr_v = ctx.enter_context(nc.sbuf_tensor([P, 2 * scr_total], f32))
        scr_a = ctx.enter_context(nc.sbuf_tensor([P, 2 * scr_total], f32))
        acc_mx = ctx.enter_context(nc.sbuf_tensor([P, S * G], f32))
        acc_b = ctx.enter_context(nc.sbuf_tensor([P, S * G], f32))
        in_sems = [ctx.enter_context(nc.semaphore(f"dma_in{s}")) for s in range(S)]
        out_sems = [ctx.enter_context(nc.semaphore(f"dma_out{i}")) for i in range(2)]
        v_sem = ctx.enter_context(nc.semaphore("v_sem"))
        a_sem = ctx.enter_context(nc.semaphore("a_sem"))
        block = ctx.enter_context(nc.Block())

        def tile_ap(se):
            return xt[:, (se % NBUF) * FREE:(se % NBUF + 1) * FREE]

        @block.sync
        def _(sync):
            for rep in range(reps):
                for s in range(S):
                    se = rep * S + s
                    if se >= NBUF:
                        sync.wait_ge(v_sem, se - NBUF + 1)
                        sync.wait_ge(a_sem, se - NBUF + 1)
                    sync.dma_start(out=tile_ap(se), in_=xv[s]).then_inc(
                        in_sems[s], 16)
            sync.wait_ge(v_sem, reps * S)
            sync.wait_ge(a_sem, reps * S)
            sync.dma_start(out=out_mx[:], in_=acc_mx[:]).then_inc(out_sems[0], 16)
            sync.dma_start(out=out_b[:], in_=acc_b[:]).then_inc(out_sems[1], 16)
            sync.wait_ge(out_sems[0], 16)
            sync.wait_ge(out_sems[1], 16)

        @block.vector
        def _(vector):
            for rep in range(reps):
                for s in range(S):
                    se = rep * S + s
                    vector.wait_ge(in_sems[s], 16 * (rep + 1))
                    if se >= 2:
                        # scratch ring region (se % 2) was last written at
                        # supertile se-2; wait for those ops to complete.
                        # (also orders accum-column reuse across reps)
                        vector.wait_ge(v_sem, se - 1)
                    t = tile_ap(se)
                    for gi, (oa, da, ob, db, cd) in enumerate(gdims):
                        col = s * G + gi
                        ins = nc.vector.scalar_tensor_tensor(
                            _view(scr_v[:], (se % 2) * scr_total + scr_off[gi], cd),
                            _view(t, oa, da),
                            0.0,
                            _view(t, ob, db),
                            mybir.AluOpType.bypass,
                            mybir.AluOpType.max,
                            accum_out=acc_mx[:, col:col + 1],
                        )
                    ins.then_inc(v_sem, 1)

        @block.scalar
        def _(scalar):
            for rep in range(reps):
                for s in range(S):
                    se = rep * S + s
                    scalar.wait_ge(in_sems[s], 16 * (rep + 1))
                    if se >= 2:
                        scalar.wait_ge(a_sem, se - 1)
                    t = tile_ap(se)
                    for gi, (oa, da, ob, db, cd) in enumerate(gdims):
                        col = s * G + gi
                        ins = nc.scalar.activation(
                            _view(scr_a[:], (se % 2) * scr_total + scr_off[gi], cd),
                            _view(t, ob, db),
                            mybir.ActivationFunctionType.Copy,
                            accum_out=acc_b[:, col:col + 1],
                        )
                    ins.then_inc(a_sem, 1)

    return nc


_CACHE = {}


def _get_program(idx_a, idx_b):
    key = (idx_a.tobytes(), idx_b.tobytes())
    if key not in _CACHE:
        _CACHE[key] = _build(_groups(idx_a, idx_b))
    return _CACHE[key]


def _make_in_maps(pred):
    shards = pred.reshape(NCORES, ROWS, F)
    return [{"x": np.ascontiguousarray(shards[i])} for i in range(NCORES)]


def kernel(**inputs):
    pred = np.ascontiguousarray(np.asarray(inputs["predictions"], dtype=np.float32))
    idx_a = np.asarray(inputs["idx_a"]).astype(np.int64)
    idx_b = np.asarray(inputs["idx_b"]).astype(np.int64)
    assert pred.shape == (B, T, F), pred.shape

    nc = _get_program(idx_a, idx_b)
    res = run_bass_kernel_spmd(nc, _make_in_maps(pred), list(range(NCORES))).results

    tot = np.float64(0.0)
    for r in res:
        tot += r["out_mx"].astype(np.float64).sum()
        tot -= r["out_b"].astype(np.float64).sum()
    return np.asarray(tot / B, dtype=np.float32)


# ---------------------------------------------------------------------------
# Benchmarking helper (test.py only; not used by the grading path).
# ---------------------------------------------------------------------------

def make_runner(np_inputs, reps=1):
    """Compile the SPMD executable once; return a zero-arg launch fn."""
    import jax
    from jax.sharding import Mesh, PartitionSpec, NamedSharding
    from jax.experimental.shard_map import shard_map
    from concourse import bass2jax
    import concourse.mybir as mb

    pred = np.ascontiguousarray(np.asarray(np_inputs["predictions"], dtype=np.float32))
    idx_a = np.asarray(np_inputs["idx_a"]).astype(np.int64)
    idx_b = np.asarray(np_inputs["idx_b"]).astype(np.int64)
    if reps == 1:
        nc = _get_program(idx_a, idx_b)
    else:
        nc = _build(_groups(idx_a, idx_b), reps=reps)
    in_maps = _make_in_maps(pred)

    bass2jax.install_neuronx_cc_hook()

    in_names, out_names, out_avals, zero_outs = [], [], [], []
    partition_name = nc.partition_id_tensor.name if nc.partition_id_tensor else None
    for alloc in nc.m.functions[0].allocations:
        if not isinstance(alloc, mb.MemoryLocationSet):
            continue
        name = alloc.memorylocations[0].name
        if alloc.kind == "ExternalInput":
            if name != partition_name:
                in_names.append(name)
        elif alloc.kind == "ExternalOutput":
            shape = tuple(alloc.tensor_shape)
            dtype = mb.dt.np(alloc.dtype)
            out_names.append(name)
            out_avals.append(jax.core.ShapedArray(shape, dtype))
            zero_outs.append(np.zeros(shape, dtype))
    n_params = len(in_names)
    n_outs = len(out_names)
    all_in_names = list(in_names) + list(out_names)
    if partition_name is not None:
        all_in_names.append(partition_name)
    donate = tuple(range(n_params, n_params + n_outs))

    def _body(*args):
        operands = list(args)
        if partition_name is not None:
            operands.append(bass2jax.partition_id_tensor())
        outs = bass2jax._bass_exec_p.bind(
            *operands,
            out_avals=tuple(out_avals),
            in_names=tuple(all_in_names),
            out_names=tuple(out_names),
            lowering_input_output_aliases=(),
            sim_require_finite=True,
            sim_require_nnan=True,
            nc=nc,
        )
        return tuple(outs)

    devices = jax.devices()[:NCORES]
    mesh = Mesh(np.asarray(devices), ("core",))
    in_specs = (PartitionSpec("core"),) * (n_params + n_outs)
    out_specs = (PartitionSpec("core"),) * n_outs
    sharded = jax.jit(
        shard_map(_body, mesh=mesh, in_specs=in_specs, out_specs=out_specs,
                  check_rep=False),
        donate_argnums=donate, keep_unused=True,
    )
    concat_in = [
        np.concatenate([np.asarray(in_maps[c][nm]) for c in range(NCORES)], axis=0)
        for nm in in_names
    ]
    sh = NamedSharding(mesh, PartitionSpec("core"))
    dev_in = [jax.device_put(a, sh) for a in concat_in]
    zeros_big = [np.zeros((NCORES * z.shape[0], *z.shape[1:]), z.dtype)
                 for z in zero_outs]

    def run_once():
        return sharded(*dev_in, *zeros_big)

    return run_once


def _time_launches(run_once, iters):
    import time
    import jax

    out = run_once()
    jax.block_until_ready(out)
    best = None
    for _ in range(3):
        t0 = time.perf_counter()
        outs = [run_once() for _ in range(iters)]
        jax.block_until_ready(outs[-1])
        t1 = time.perf_counter()
        per = (t1 - t0) / iters
        best = per if best is None else min(best, per)
    return best


def measure_hw_ns(np_inputs, iters=32, reps_hi=17):
    """Device-side kernel time via the K-reps delta method.

    The bench NEFF repeats the whole pipeline K times back-to-back; the
    difference in per-launch wall time between K=reps_hi and K=1 divided
    by (reps_hi-1) cancels all host/network dispatch overhead.
    """
    t1 = _time_launches(make_runner(np_inputs, reps=1), iters)
    th = _time_launches(make_runner(np_inputs, reps=reps_hi), iters)
    return int((th - t1) / (reps_hi - 1) * 1e9)
